# revision 86
# baseline (speedup 1.0000x reference)
"""Multi-head attention (multiquery K/V) Bass kernel for 8 trn2 NeuronCores.

Sharding: 8 cores = 2 batches x 4 query-row quarters. Each core computes the
full multiquery K/V projections for its batch (cheap, dk=64) and runs
attention + output projection for its 512 query rows over all 16 heads.
Output is a pure concatenation across cores -- no collectives.

Design (v3):
- The Scalar engine's exp over [t=2048, s=512] x 16 heads (~135us at
  1 elem/cycle/lane) is the per-core floor; everything else hides under it.
- Every steady-state matmul runs in the PE's default (128,128) mode so the
  array never drains for a tiling-mode switch:
  * scores use the twice-stacked K (K2T rows 0:64 == 64:128 == K.T) against
    zero-padded per-head Q slices (qz[j=0] = [Q_even; 0], qz[j=1] =
    [0; Q_odd]), making the contraction a full 128 rows;
  * attn@V keeps t=128 contraction with a [1|V] stationary of width 65 whose
    ones column accumulates the softmax denominator into psum row 0;
  * the fused output projection contracts the head pair (128 rows).
- 8 passes of one head pair each. PSUM: sc double buffer (4 banks) + attn@V
  accumulator (2 banks) + two 1-bank aux slots = 8 banks.
- Normalize: reciprocal_approx_fast of psum row 0 (the custom-DVE op ignores
  AP partition offsets on HW, so the denominator must live at partition 0)
  into row 0 of a zeroed [65,2,512] tile; a ones[65,65]-stationary matmul
  broadcasts it across partitions; DVE multiplies write the pair-stacked oT
  (odd head to SBUF partitions 64:128). Normalize for pass P runs before
  pass P+1's first attn@V so the accumulator hand-off never stalls exp.
- Projections for x-blocks 1..3 / q-blocks 1..7 are emitted as hooks inside
  early passes, filling PE slack under the exp cadence.
- dma_start costs ~1us of GpSimd issue time each, so only the 5 transfers
  needed by the pre-pass are issued first; the rest issue behind them.
"""

import sys

import numpy as np

if "/opt/trn_rl_repo" not in sys.path:
    sys.path.insert(0, "/opt/trn_rl_repo")

B, S, D = 2, 2048, 1024
H, DK = 16, 64
H2 = H // 2  # head pairs
P = 128
NCORES, GPB = 8, 4
SPB = S // GPB  # 512 query rows per core
KC = D // P  # 8 contraction subtiles over d_model
NT = S // P  # 16 key/t blocks
NSB = SPB // P  # 4 s blocks


def build_bass(scale: float, debug: bool = False):
    import concourse.bacc as bacc
    import concourse.mybir as mybir
    import concourse.tile as tile
    from concourse.bass import ts
    from concourse.dve_ops import (
        RECIP_APPROX_FAST_CONSTS,
        RECIPROCAL_APPROX_FAST,
    )

    fp32 = mybir.dt.float32
    mdt = mybir.dt.float32r  # fp32 bits, streams 1 cycle/row on the PE
    Act = mybir.ActivationFunctionType

    bf16 = mybir.dt.bfloat16
    nc = bacc.Bacc(None, target_bir_lowering=False)
    xT = nc.dram_tensor("xT", [D, S], bf16, kind="ExternalInput")
    cst = nc.dram_tensor("cst", [P, 256], mdt, kind="ExternalInput")
    wqT = nc.dram_tensor("wqT", [D, D], bf16, kind="ExternalInput")
    wkkT = nc.dram_tensor("wkkT", [D, P], bf16, kind="ExternalInput")
    wvT = nc.dram_tensor("wvT", [D, DK + 1], bf16, kind="ExternalInput")
    wo2 = nc.dram_tensor("wo2", [P, H2, D], mdt, kind="ExternalInput")
    y = nc.dram_tensor("y", [SPB, D], bf16, kind="ExternalOutput")
    if debug:
        dacc = nc.dram_tensor("dacc", [DK + 1, 2, SPB], fp32,
                              kind="ExternalOutput")
        doT = nc.dram_tensor("doT", [H2, P, SPB], fp32, kind="ExternalOutput")

    xT3 = xT.rearrange("(po pi) s -> pi po s", pi=P)
    wq3 = wqT.rearrange("(po pi) d -> pi po d", pi=P)
    wkk3 = wkkT.rearrange("(po pi) d -> pi po d", pi=P)
    wv3 = wvT.rearrange("(po pi) d -> pi po d", pi=P)

    with tile.TileContext(nc) as tc:
        with (
            tc.tile_pool(name="sb", bufs=1) as sb,
            tc.tile_pool(name="ps", bufs=1, space="PSUM") as ps,
        ):
            # ---- persistent SBUF ----
            cst_sb = sb.tile([P, 256], mdt, name="cst")
            K2T = sb.tile([P, S], bf16, name="K2T")
            # Vp stationary is 96 wide: col 0 = denominator ones column,
            # cols 32:96 = V', so attn@V lands den at psum row 0 (readable by
            # the offset-dropping custom reciprocal) and V at the aligned
            # rows 32:96
            Vp = sb.tile([P, NT, 96], mdt, name="Vp")
            qz = sb.tile([P, KC, 2, SPB], bf16, name="qz")
            rec65 = sb.tile([DK + 1, 2, SPB], mdt, name="rec65")
            acc_sb = sb.tile([96, 2, SPB], fp32, name="acc_sb")
            y_sb = sb.tile([P, NSB, D], fp32, name="y_sb")
            wkk_sb = sb.tile([P, KC, P], bf16, name="wkk")
            wv_sb = sb.tile([P, KC, DK + 1], bf16, name="wv")
            wq_sb = sb.tile([P, KC, D], bf16, name="wq")
            wo2_sb = sb.tile([P, H2, D], mdt, name="wo2")
            xc0 = sb.tile([P, KC, SPB], bf16, name="xc0")

            ident = cst_sb[0:DK, 0:DK]
            ones96 = cst_sb[0:DK + 1, DK:DK + 96]  # [65, 96] of ones

            # ---- critical-path DMAs (pre-pass needs only these; xc0 is the
            # long pole so its halves issue first) ----
            nc.gpsimd.dma_start(xc0[:], xT3[:, :, 0:SPB])
            nc.gpsimd.dma_start(cst_sb[:], cst[:])
            nc.gpsimd.dma_start(wkk_sb[:], wkk3[:])
            nc.gpsimd.dma_start(wv_sb[:], wv3[:])
            nc.gpsimd.dma_start(wq_sb[:, :, ts(0, P)], wq3[:, :, ts(0, P)])
            nc.vector.memzero(rec65[:])
            nc.vector.memzero(qz[:])
            nc.vector.memzero(Vp[:])

            xcs = {0: xc0}

            def aux(shape, dtype=fp32):
                return ps.tile(shape, dtype, name="aux", tag="aux", bufs=2)

            # projection emitters, split into <=1us pieces so they slot into
            # per-tb PE slack without stalling the exp cadence; piece "a"
            # starts the psum accumulation, "b" finishes it and copies out
            pstate = {}

            def k2_a(c, half=None):
                k2ps = aux([P, 512])
                pstate[("k2", c)] = k2ps
                for k in range(4):
                    nc.tensor.matmul(
                        k2ps[:], wkk_sb[:, k, :], xcs[c][:, k, :],
                        start=(k == 0), stop=False,
                    )

            def k2_b(c):
                k2ps = pstate.pop(("k2", c))
                for k in range(4, KC):
                    nc.tensor.matmul(
                        k2ps[:], wkk_sb[:, k, :], xcs[c][:, k, :],
                        start=False, stop=(k == KC - 1),
                    )
                nc.vector.tensor_copy(K2T[:, ts(c, 512)], k2ps[:])

            def v_a(c):
                vps = aux([DK + 1, 512])
                pstate[("v", c)] = vps
                for k in range(4):
                    nc.tensor.matmul(
                        vps[:], wv_sb[:, k, :], xcs[c][:, k, :],
                        start=(k == 0), stop=False,
                    )

            def v_b(c):
                vps = pstate.pop(("v", c))
                for k in range(4, KC):
                    nc.tensor.matmul(
                        vps[:], wv_sb[:, k, :], xcs[c][:, k, :],
                        start=False, stop=(k == KC - 1),
                    )
                vsb = sb.tile([DK, SPB], mdt, name="vsb", tag="vsb", bufs=1)
                pstate[("vsb", c)] = vsb
                nc.vector.tensor_copy(vsb[:], vps[0:DK, :])

            def v_tr(c):
                # PE-transpose V.T -> V' [t, dv] into Vp cols 0:64 (col 64
                # is the denominator ones column)
                vsb = pstate.pop(("vsb", c))
                for pair in range(2):
                    trs = [aux([P, DK], mdt), aux([P, DK], mdt)]
                    for q in range(2):
                        nc.tensor.transpose(
                            trs[q][:], vsb[:, ts(2 * pair + q, P)], ident
                        )
                    for q in range(2):
                        nc.vector.tensor_copy(
                            Vp[:, 4 * c + 2 * pair + q, 32:96], trs[q][:]
                        )

            def q_a(m):
                qps = aux([P, 512])
                pstate[("q", m)] = qps
                for k in range(4):
                    nc.tensor.matmul(
                        qps[:], wq_sb[:, k, ts(m, P)], xc0[:, k, :],
                        start=(k == 0), stop=False,
                    )

            def q_b(m):
                qps = pstate.pop(("q", m))
                for k in range(4, KC):
                    nc.tensor.matmul(
                        qps[:], wq_sb[:, k, ts(m, P)], xc0[:, k, :],
                        start=False, stop=(k == KC - 1),
                    )
                # qz[j=0] = [Q_even; 0], qz[j=1] = [0; Q_odd] (zero-padded at
                # build start) so scores contract a full 128 rows
                nc.vector.tensor_copy(qz[0:DK, m, 0, :], qps[0:DK, :])
                nc.vector.tensor_copy(qz[DK:P, m, 1, :], qps[DK:P, :])

            # ---- remaining DMAs (xc1 feeds the pre-pass; the rest stream
            # in behind it, ordered by first use) ----
            xc1 = sb.tile([P, KC, SPB], bf16, name="xc", tag="xc", bufs=2)
            nc.gpsimd.dma_start(xc1[:], xT3[:, :, ts(1, SPB)])
            xcs[1] = xc1
            nc.gpsimd.dma_start(wq_sb[:, :, ts(1, P)], wq3[:, :, ts(1, P)])
            nc.gpsimd.dma_start(Vp[:, :, 0], cst[:, DK:DK + NT])  # ones col
            xc2 = sb.tile([P, KC, SPB], bf16, name="xc", tag="xc", bufs=2)
            nc.gpsimd.dma_start(xc2[:], xT3[:, :, ts(2, SPB)])
            xcs[2] = xc2
            xc3 = sb.tile([P, KC, SPB], bf16, name="xc", tag="xc", bufs=2)
            nc.gpsimd.dma_start(xc3[:], xT3[:, :, ts(3, SPB)])
            xcs[3] = xc3
            nc.gpsimd.dma_start(wo2_sb[:, 0, :], wo2[:, 0, :])
            nc.gpsimd.dma_start(wq_sb[:, :, ts(2, P)], wq3[:, :, ts(2, P)])
            nc.gpsimd.dma_start(wo2_sb[:, 1, :], wo2[:, 1, :])
            for m in range(3, KC):
                nc.gpsimd.dma_start(wq_sb[:, :, ts(m, P)], wq3[:, :, ts(m, P)])
            for hp in range(2, H2):
                nc.gpsimd.dma_start(wo2_sb[:, hp, :], wo2[:, hp, :])

            # ---- pre-pass: K2/V for c0 and Q for m0, m1 ----
            k2_a(0)
            k2_b(0)
            q_a(0)
            q_b(0)
            q_a(1)
            q_b(1)
            v_a(0)
            v_b(0)
            v_tr(0)

            # ---- attention passes, one head pair each ----
            def emit_av(acc, tb, ex):
                first, last = (tb == 0), (tb == NT - 1)
                for j in range(2):
                    nc.tensor.matmul(
                        acc[:, j, :], Vp[:, tb, :], ex[:, j, :],
                        start=first, stop=last,
                    )

            def norm_a(prev_hp):
                # reciprocal of the denominator row (psum row 0 -> acc_sb
                # row 0, the tile base, so the offset-dropping custom op
                # reads the right partition)
                c = RECIP_APPROX_FAST_CONSTS
                nc.vector._custom_dve(
                    RECIPROCAL_APPROX_FAST,
                    out=rec65[0:1, :, :],
                    in0=acc_sb[0:1, :, :],
                    s0=c["s0"], s1=c["s1"], imm2=c["imm2"],
                )

            def norm_b(prev_hp, tail=False):
                # broadcast 1/den across partitions (ones-stationary matmul)
                # and write the pair-stacked normalized output oT.  In the
                # tail the Scalar engine is idle (no more exps), so the bc
                # evacuation runs there, off the serial DVE chain.
                oT = sb.tile([P, SPB], mdt, name="oT", tag="oT", bufs=2)
                for j in range(2):
                    bc = aux([96, SPB])
                    nc.tensor.matmul(
                        bc[:], ones96, rec65[:, j, :], start=True, stop=True
                    )
                    bcs = sb.tile([96, SPB], fp32, name="bcs", tag="vsb",
                                  bufs=1)
                    if tail:
                        nc.scalar.copy(bcs[:], bc[:])
                    else:
                        nc.vector.tensor_copy(bcs[:], bc[:])
                    # 32-row pieces: spans starting at partition 32/96 may
                    # cover at most 32 partitions, and SBUF+SBUF inputs must
                    # share a base partition
                    for hf in range(2):
                        nc.vector.tensor_mul(
                            oT[j * DK + hf * 32:j * DK + hf * 32 + 32, :],
                            acc_sb[32 + hf * 32:64 + hf * 32, j, :],
                            bcs[32 + hf * 32:64 + hf * 32, :],
                        )
                if debug:
                    nc.gpsimd.dma_start(dacc[:], acc_sb[0:DK + 1, :, :])
                    nc.gpsimd.dma_start(doT[prev_hp, :, :], oT[:])
                return oT

            def emit_y(prev_hp, oT, sb4):
                yps = [aux([P, 512]), aux([P, 512])]
                for df in range(2):
                    nc.tensor.matmul(
                        yps[df][:], oT[:, ts(sb4, P)],
                        wo2_sb[:, prev_hp, ts(df, 512)],
                        start=True, stop=True,
                    )
                for df in range(2):
                    if prev_hp == 0:
                        nc.vector.tensor_copy(
                            y_sb[:, sb4, ts(df, 512)], yps[df][:]
                        )
                    else:
                        nc.vector.tensor_add(
                            y_sb[:, sb4, ts(df, 512)], yps[df][:],
                            y_sb[:, sb4, ts(df, 512)],
                        )

            def q_p(m, i):
                if i == 0:
                    pstate[("q", m)] = aux([P, 512])
                qps = pstate[("q", m)]
                for k in (2 * i, 2 * i + 1):
                    nc.tensor.matmul(
                        qps[:], wq_sb[:, k, ts(m, P)], xc0[:, k, :],
                        start=(k == 0), stop=(k == KC - 1),
                    )

            def q_fin(m):
                qps = pstate.pop(("q", m))
                nc.vector.tensor_copy(qz[0:DK, m, 0, :], qps[0:DK, :])
                nc.vector.tensor_copy(qz[DK:P, m, 1, :], qps[DK:P, :])

            # hook schedule: {pass: {tb: [closures]}} -- one <=1us piece per
            # tb so the PE never bursts past the exp cadence.  Deadlines:
            # K2T block c by sc(4c) emission; Vp block c by av(4c) (tb 4c+4,
            # attn@V lag 4); qz m by the next pass's sc(0).
            hooks = {
                0: {1: [lambda: k2_a(1)], 2: [lambda: k2_b(1)],
                    3: [lambda: v_a(1)], 4: [lambda: v_b(1)],
                    5: [lambda: v_tr(1)],
                    6: [lambda: k2_a(2)], 7: [lambda: k2_b(2)],
                    8: [lambda: v_a(2)], 9: [lambda: v_b(2)],
                    10: [lambda: v_tr(2), lambda: k2_a(3)],
                    11: [lambda: k2_b(3)],
                    12: [lambda: v_a(3)], 13: [lambda: v_b(3)],
                    14: [lambda: v_tr(3)]},
            }
            for p in range(1, 7):
                for i in range(4):
                    hooks.setdefault(p, {}).setdefault(10 + i, []).append(
                        lambda m=p + 1, i=i: q_p(m, i)
                    )
                hooks.setdefault(p, {}).setdefault(14, []).append(
                    lambda m=p + 1: q_fin(m)
                )

            prev = None  # (hp, acc, tail exs 13..15)
            for hp in range(H2):
                acc = ps.tile([96, 2, SPB], fp32, name="acc", tag="acc",
                              bufs=1)
                exs = {}
                oT_prev = None
                for tb in range(NT):
                    sc = ps.tile([P, 2, SPB], fp32, name=f"sc{tb % 2}",
                                 tag=f"sc{tb % 2}", bufs=1)
                    for j in range(2):
                        nc.tensor.matmul(
                            sc[:, j, :], K2T[:, ts(tb, P)], qz[:, hp, j, :],
                            start=True, stop=True,
                        )
                    # attn@V runs 4 tb behind exp (ex bufs=4); emitted before
                    # the activation so the freed ex slot is ready in time.
                    # The previous pass's last four attn@V are spread over
                    # its own tb15 (av 12) and this pass's tb0/tb1 so no
                    # single tb block overloads the PE.
                    if tb == 0 and prev is not None:
                        emit_av(prev[1], 13, prev[2].pop(13))
                    if tb == 1 and prev is not None:
                        emit_av(prev[1], 14, prev[2].pop(14))
                    if tb >= 4:
                        emit_av(acc, tb - 4, exs.pop(tb - 4))
                    ex = sb.tile([P, 2, SPB], mdt, name="ex", tag="ex", bufs=4)
                    nc.scalar.activation(ex[:], sc[:], Act.Exp, scale=scale)
                    exs[tb] = ex
                    if tb == 1 and prev is not None:
                        emit_av(prev[1], 15, prev[2].pop(15))
                        # evacuate the finished accumulator so this pass's
                        # attn@V can claim the psum banks at tb4; the
                        # normalize then works from the SBUF copy at leisure
                        nc.vector.tensor_copy(acc_sb[:], prev[1][:])
                    if tb == 15:
                        emit_av(acc, 12, exs.pop(12))
                    if prev is not None:
                        if tb == 2:
                            norm_a(prev[0])
                        elif tb == 3:
                            oT_prev = norm_b(prev[0])
                        elif 7 <= tb <= 10:
                            emit_y(prev[0], oT_prev, tb - 7)
                    for fn in hooks.get(hp, {}).get(tb, []):
                        fn()
                prev = (hp, acc, exs)

            # tail: last pass's deferred attn@V + normalize + y + writeback.
            # y partials rotate through the dead sc banks as well as aux so
            # the four output blocks pipeline instead of serializing; the
            # final adds cast to a bf16 output buffer, halving writeback DMA.
            y_out = sb.tile([P, NSB, D], bf16, name="y_out")
            for tb in range(13, NT):
                emit_av(prev[1], tb, prev[2].pop(tb))
            # tail normalize: only rows 0:32 (which hold the denominator)
            # gate the reciprocal; the V rows follow while it runs
            nc.vector.tensor_copy(acc_sb[0:32, :, :], prev[1][0:32, :, :])
            norm_a(prev[0])
            nc.vector.tensor_copy(
                acc_sb[32:64, :, :], prev[1][32:64, :, :]
            )
            nc.vector.tensor_copy(
                acc_sb[DK:96, :, :], prev[1][DK:96, :, :]
            )
            oT_last = norm_b(prev[0], tail=True)
            for sb4 in range(NSB):
                yps = [
                    ps.tile([P, 512], fp32, name="typs", tag=f"sc{sb4 % 2}",
                            bufs=1),
                    aux([P, 512]),
                ]
                for df in range(2):
                    nc.tensor.matmul(
                        yps[df][:], oT_last[:, ts(sb4, P)],
                        wo2_sb[:, H2 - 1, ts(df, 512)],
                        start=True, stop=True,
                    )
                for df in range(2):
                    nc.vector.tensor_add(
                        y_out[:, sb4, ts(df, 512)], yps[df][:],
                        y_sb[:, sb4, ts(df, 512)],
                    )
                # issue from the Scalar engine: idle after the last exp, and
                # far cheaper per issue than gpsimd
                nc.scalar.dma_start(y[ts(sb4, P), :], y_out[:, sb4, :])

    nc.compile()
    return nc


def make_in_maps(x, w_q, w_k, w_v, w_out):
    import ml_dtypes

    bf16 = ml_dtypes.bfloat16
    cst = np.zeros((P, 256), dtype=np.float32)
    cst[0:DK, 0:DK] = np.eye(DK, dtype=np.float32)
    cst[:, DK:192] = 1.0
    x = np.ascontiguousarray(np.asarray(x, dtype=np.float32))
    w_q = np.asarray(w_q, dtype=np.float32)
    w_k = np.asarray(w_k, dtype=np.float32)
    w_v = np.asarray(w_v, dtype=np.float32)
    w_out = np.asarray(w_out, dtype=np.float32)

    wqT = np.ascontiguousarray(w_q.T.astype(bf16))
    wkkT = np.ascontiguousarray(
        np.concatenate([w_k.T, w_k.T], axis=1).astype(bf16)
    )
    wvT = np.ascontiguousarray(
        np.concatenate([w_v.T, np.zeros((D, 1), np.float32)], axis=1)
        .astype(bf16)
    )
    # head-pair-stacked w_out.T: wo2[phi*64+dv, hp, d] = w_out.T[(2hp+phi)*64+dv, d]
    wo2 = np.ascontiguousarray(
        w_out.T.reshape(H2, 2, DK, D).transpose(1, 2, 0, 3).reshape(P, H2, D)
    )

    in_maps = []
    for c in range(NCORES):
        b, r = divmod(c, GPB)
        # roll this core's query rows to the front; t-order is irrelevant
        # (attention sums over t), so K/V are unaffected
        xb = np.roll(x[b], -r * SPB, axis=0)
        xTc = np.ascontiguousarray(xb.T.astype(bf16))
        in_maps.append(
            {"xT": xTc, "wqT": wqT, "wkkT": wkkT, "wvT": wvT, "wo2": wo2,
             "cst": cst}
        )
    return in_maps


_BUILD_CACHE = {}


def _cached_nc(scale: float):
    key = round(float(scale), 12)
    if key not in _BUILD_CACHE:
        _BUILD_CACHE[key] = build_bass(float(scale))
    return _BUILD_CACHE[key]


def run_on_hw(in_maps, scale, trace=False):
    from concourse.bass_utils import run_bass_kernel_spmd

    nc = _cached_nc(scale)
    return run_bass_kernel_spmd(nc, in_maps, list(range(NCORES)), trace=trace)


def assemble(results):
    out = np.empty((B, S, D), dtype=np.float32)
    for c in range(NCORES):
        b, r = divmod(c, GPB)
        out[b, r * SPB:(r + 1) * SPB] = results[c]["y"].astype(np.float32)
    return out


def kernel(x, w_q, w_k, w_v, w_out, softmax_scale):
    scale = float(np.asarray(softmax_scale).reshape(-1)[0])
    in_maps = make_in_maps(x, w_q, w_k, w_v, w_out)
    res = run_on_hw(in_maps, scale, trace=False)
    return assemble(res.results)


# revision 87
# speedup vs baseline: 189.0414x; 189.0414x over previous
"""Multi-head attention (multiquery K/V) Bass kernel for 8 trn2 NeuronCores.

Sharding: 8 cores = 2 batches x 4 query-row quarters. Each core computes the
full multiquery K/V projections for its batch (cheap, dk=64) and runs
attention + output projection for its 512 query rows over all 16 heads.
Output is a pure concatenation across cores -- no collectives.

Design (v3):
- The Scalar engine's exp over [t=2048, s=512] x 16 heads (~135us at
  1 elem/cycle/lane) is the per-core floor; everything else hides under it.
- Every steady-state matmul runs in the PE's default (128,128) mode so the
  array never drains for a tiling-mode switch:
  * scores use the twice-stacked K (K2T rows 0:64 == 64:128 == K.T) against
    zero-padded per-head Q slices (qz[j=0] = [Q_even; 0], qz[j=1] =
    [0; Q_odd]), making the contraction a full 128 rows;
  * attn@V keeps t=128 contraction with a [1|V] stationary of width 65 whose
    ones column accumulates the softmax denominator into psum row 0;
  * the fused output projection contracts the head pair (128 rows).
- 8 passes of one head pair each. PSUM: sc double buffer (4 banks) + attn@V
  accumulator (2 banks) + two 1-bank aux slots = 8 banks.
- Normalize: reciprocal_approx_fast of psum row 0 (the custom-DVE op ignores
  AP partition offsets on HW, so the denominator must live at partition 0)
  into row 0 of a zeroed [65,2,512] tile; a ones[65,65]-stationary matmul
  broadcasts it across partitions; DVE multiplies write the pair-stacked oT
  (odd head to SBUF partitions 64:128). Normalize for pass P runs before
  pass P+1's first attn@V so the accumulator hand-off never stalls exp.
- Projections for x-blocks 1..3 / q-blocks 1..7 are emitted as hooks inside
  early passes, filling PE slack under the exp cadence.
- dma_start costs ~1us of GpSimd issue time each, so only the 5 transfers
  needed by the pre-pass are issued first; the rest issue behind them.
"""

import sys

import numpy as np

if "/opt/trn_rl_repo" not in sys.path:
    sys.path.insert(0, "/opt/trn_rl_repo")

B, S, D = 2, 2048, 1024
H, DK = 16, 64
H2 = H // 2  # head pairs
P = 128
NCORES, GPB = 8, 4
SPB = S // GPB  # 512 query rows per core
KC = D // P  # 8 contraction subtiles over d_model
NT = S // P  # 16 key/t blocks
NSB = SPB // P  # 4 s blocks


def build_bass(scale: float, debug: bool = False):
    import concourse.bacc as bacc
    import concourse.mybir as mybir
    import concourse.tile as tile
    from concourse.bass import ts
    from concourse.dve_ops import (
        RECIP_APPROX_FAST_CONSTS,
        RECIPROCAL_APPROX_FAST,
    )

    fp32 = mybir.dt.float32
    mdt = mybir.dt.float32r  # fp32 bits, streams 1 cycle/row on the PE
    Act = mybir.ActivationFunctionType

    bf16 = mybir.dt.bfloat16
    nc = bacc.Bacc(None, target_bir_lowering=False)
    xT = nc.dram_tensor("xT", [D, S], bf16, kind="ExternalInput")
    cst = nc.dram_tensor("cst", [P, 256], mdt, kind="ExternalInput")
    wqT = nc.dram_tensor("wqT", [D, D], bf16, kind="ExternalInput")
    wkkT = nc.dram_tensor("wkkT", [D, P], bf16, kind="ExternalInput")
    wvT = nc.dram_tensor("wvT", [D, DK + 1], bf16, kind="ExternalInput")
    wo2 = nc.dram_tensor("wo2", [P, H2, D], mdt, kind="ExternalInput")
    y = nc.dram_tensor("y", [SPB, D], bf16, kind="ExternalOutput")
    if debug:
        dacc = nc.dram_tensor("dacc", [DK + 1, 2, SPB], fp32,
                              kind="ExternalOutput")
        doT = nc.dram_tensor("doT", [H2, P, SPB], fp32, kind="ExternalOutput")

    xT3 = xT.rearrange("(po pi) s -> pi po s", pi=P)
    wq3 = wqT.rearrange("(po pi) d -> pi po d", pi=P)
    wkk3 = wkkT.rearrange("(po pi) d -> pi po d", pi=P)
    wv3 = wvT.rearrange("(po pi) d -> pi po d", pi=P)

    with tile.TileContext(nc) as tc:
        with (
            tc.tile_pool(name="sb", bufs=1) as sb,
            tc.tile_pool(name="ps", bufs=1, space="PSUM") as ps,
        ):
            # ---- persistent SBUF ----
            cst_sb = sb.tile([P, 256], mdt, name="cst")
            K2T = sb.tile([P, S], bf16, name="K2T")
            # Vp stationary is 96 wide: col 0 = denominator ones column,
            # cols 32:96 = V', so attn@V lands den at psum row 0 (readable by
            # the offset-dropping custom reciprocal) and V at the aligned
            # rows 32:96
            Vp = sb.tile([P, NT, 96], mdt, name="Vp")
            qz = sb.tile([P, KC, 2, SPB], bf16, name="qz")
            rec65 = sb.tile([DK + 1, 2, SPB], mdt, name="rec65")
            acc_sb = sb.tile([96, 2, SPB], fp32, name="acc_sb")
            y_sb = sb.tile([P, NSB, D], fp32, name="y_sb")
            wkk_sb = sb.tile([P, KC, P], bf16, name="wkk")
            wv_sb = sb.tile([P, KC, DK + 1], bf16, name="wv")
            wq_sb = sb.tile([P, KC, D], bf16, name="wq")
            wo2_sb = sb.tile([P, H2, D], mdt, name="wo2")
            xc0 = sb.tile([P, KC, SPB], bf16, name="xc0")

            ident = cst_sb[0:DK, 0:DK]
            ones96 = cst_sb[0:DK + 1, DK:DK + 96]  # [65, 96] of ones

            # ---- critical-path DMAs (pre-pass needs only these; xc0 is the
            # long pole so its halves issue first) ----
            nc.gpsimd.dma_start(xc0[:], xT3[:, :, 0:SPB])
            nc.gpsimd.dma_start(cst_sb[:], cst[:])
            nc.gpsimd.dma_start(wkk_sb[:], wkk3[:])
            nc.gpsimd.dma_start(wv_sb[:], wv3[:])
            nc.gpsimd.dma_start(wq_sb[:, :, ts(0, P)], wq3[:, :, ts(0, P)])
            nc.vector.memzero(rec65[:])
            nc.vector.memzero(qz[:])
            nc.vector.memzero(Vp[:])

            xcs = {0: xc0}

            def aux(shape, dtype=fp32):
                return ps.tile(shape, dtype, name="aux", tag="aux", bufs=2)

            # projection emitters, split into <=1us pieces so they slot into
            # per-tb PE slack without stalling the exp cadence; piece "a"
            # starts the psum accumulation, "b" finishes it and copies out
            pstate = {}

            def k2_a(c, half=None):
                k2ps = aux([P, 512])
                pstate[("k2", c)] = k2ps
                for k in range(4):
                    nc.tensor.matmul(
                        k2ps[:], wkk_sb[:, k, :], xcs[c][:, k, :],
                        start=(k == 0), stop=False,
                    )

            def k2_b(c):
                k2ps = pstate.pop(("k2", c))
                for k in range(4, KC):
                    nc.tensor.matmul(
                        k2ps[:], wkk_sb[:, k, :], xcs[c][:, k, :],
                        start=False, stop=(k == KC - 1),
                    )
                nc.vector.tensor_copy(K2T[:, ts(c, 512)], k2ps[:])

            def v_a(c):
                vps = aux([DK + 1, 512])
                pstate[("v", c)] = vps
                for k in range(4):
                    nc.tensor.matmul(
                        vps[:], wv_sb[:, k, :], xcs[c][:, k, :],
                        start=(k == 0), stop=False,
                    )

            def v_b(c):
                vps = pstate.pop(("v", c))
                for k in range(4, KC):
                    nc.tensor.matmul(
                        vps[:], wv_sb[:, k, :], xcs[c][:, k, :],
                        start=False, stop=(k == KC - 1),
                    )
                vsb = sb.tile([DK, SPB], mdt, name="vsb", tag="vsb", bufs=1)
                pstate[("vsb", c)] = vsb
                nc.vector.tensor_copy(vsb[:], vps[0:DK, :])

            def v_tr(c):
                # PE-transpose V.T -> V' [t, dv] into Vp cols 0:64 (col 64
                # is the denominator ones column)
                vsb = pstate.pop(("vsb", c))
                for pair in range(2):
                    trs = [aux([P, DK], mdt), aux([P, DK], mdt)]
                    for q in range(2):
                        nc.tensor.transpose(
                            trs[q][:], vsb[:, ts(2 * pair + q, P)], ident
                        )
                    for q in range(2):
                        nc.vector.tensor_copy(
                            Vp[:, 4 * c + 2 * pair + q, 32:96], trs[q][:]
                        )

            def q_a(m):
                qps = aux([P, 512])
                pstate[("q", m)] = qps
                for k in range(4):
                    nc.tensor.matmul(
                        qps[:], wq_sb[:, k, ts(m, P)], xc0[:, k, :],
                        start=(k == 0), stop=False,
                    )

            def q_b(m):
                qps = pstate.pop(("q", m))
                for k in range(4, KC):
                    nc.tensor.matmul(
                        qps[:], wq_sb[:, k, ts(m, P)], xc0[:, k, :],
                        start=False, stop=(k == KC - 1),
                    )
                # qz[j=0] = [Q_even; 0], qz[j=1] = [0; Q_odd] (zero-padded at
                # build start) so scores contract a full 128 rows
                nc.vector.tensor_copy(qz[0:DK, m, 0, :], qps[0:DK, :])
                nc.vector.tensor_copy(qz[DK:P, m, 1, :], qps[DK:P, :])

            # ---- remaining DMAs (xc1 feeds the pre-pass; the rest stream
            # in behind it, ordered by first use) ----
            xc1 = sb.tile([P, KC, SPB], bf16, name="xc", tag="xc", bufs=2)
            nc.gpsimd.dma_start(xc1[:], xT3[:, :, ts(1, SPB)])
            xcs[1] = xc1
            nc.gpsimd.dma_start(wq_sb[:, :, ts(1, P)], wq3[:, :, ts(1, P)])
            nc.gpsimd.dma_start(Vp[:, :, 0], cst[:, DK:DK + NT])  # ones col
            xc2 = sb.tile([P, KC, SPB], bf16, name="xc", tag="xc", bufs=2)
            nc.gpsimd.dma_start(xc2[:], xT3[:, :, ts(2, SPB)])
            xcs[2] = xc2
            xc3 = sb.tile([P, KC, SPB], bf16, name="xc", tag="xc", bufs=2)
            nc.gpsimd.dma_start(xc3[:], xT3[:, :, ts(3, SPB)])
            xcs[3] = xc3
            nc.gpsimd.dma_start(wo2_sb[:, 0, :], wo2[:, 0, :])
            nc.gpsimd.dma_start(wq_sb[:, :, ts(2, P)], wq3[:, :, ts(2, P)])
            nc.gpsimd.dma_start(wo2_sb[:, 1, :], wo2[:, 1, :])
            for m in range(3, KC):
                nc.gpsimd.dma_start(wq_sb[:, :, ts(m, P)], wq3[:, :, ts(m, P)])
            for hp in range(2, H2):
                nc.gpsimd.dma_start(wo2_sb[:, hp, :], wo2[:, hp, :])

            # ---- pre-pass: K2/V for c0 and Q for m0, m1 ----
            k2_a(0)
            k2_b(0)
            q_a(0)
            q_b(0)
            q_a(1)
            q_b(1)
            v_a(0)
            v_b(0)
            v_tr(0)

            # ---- attention passes, one head pair each ----
            def emit_av(acc, tb, ex):
                first, last = (tb == 0), (tb == NT - 1)
                for j in range(2):
                    nc.tensor.matmul(
                        acc[:, j, :], Vp[:, tb, :], ex[:, j, :],
                        start=first, stop=last,
                    )

            def norm_a(prev_hp):
                # reciprocal of the denominator row (psum row 0 -> acc_sb
                # row 0, the tile base, so the offset-dropping custom op
                # reads the right partition)
                c = RECIP_APPROX_FAST_CONSTS
                nc.vector._custom_dve(
                    RECIPROCAL_APPROX_FAST,
                    out=rec65[0:1, :, :],
                    in0=acc_sb[0:1, :, :],
                    s0=c["s0"], s1=c["s1"], imm2=c["imm2"],
                )

            def norm_b(prev_hp, tail=False):
                # broadcast 1/den across partitions (ones-stationary matmul)
                # and write the pair-stacked normalized output oT.  In the
                # tail the Scalar engine is idle (no more exps), so the bc
                # evacuation runs there, off the serial DVE chain.
                oT = sb.tile([P, SPB], mdt, name="oT", tag="oT", bufs=2)
                for j in range(2):
                    bc = aux([96, SPB])
                    nc.tensor.matmul(
                        bc[:], ones96, rec65[:, j, :], start=True, stop=True
                    )
                    bcs = sb.tile([96, SPB], fp32, name="bcs", tag="vsb",
                                  bufs=1)
                    nc.vector.tensor_copy(bcs[:], bc[:])
                    # 32-row pieces: spans starting at partition 32/96 may
                    # cover at most 32 partitions, and SBUF+SBUF inputs must
                    # share a base partition
                    for hf in range(2):
                        nc.vector.tensor_mul(
                            oT[j * DK + hf * 32:j * DK + hf * 32 + 32, :],
                            acc_sb[32 + hf * 32:64 + hf * 32, j, :],
                            bcs[32 + hf * 32:64 + hf * 32, :],
                        )
                if debug:
                    nc.gpsimd.dma_start(dacc[:], acc_sb[0:DK + 1, :, :])
                    nc.gpsimd.dma_start(doT[prev_hp, :, :], oT[:])
                return oT

            def emit_y(prev_hp, oT, sb4):
                yps = [aux([P, 512]), aux([P, 512])]
                for df in range(2):
                    nc.tensor.matmul(
                        yps[df][:], oT[:, ts(sb4, P)],
                        wo2_sb[:, prev_hp, ts(df, 512)],
                        start=True, stop=True,
                    )
                for df in range(2):
                    if prev_hp == 0:
                        nc.vector.tensor_copy(
                            y_sb[:, sb4, ts(df, 512)], yps[df][:]
                        )
                    else:
                        nc.vector.tensor_add(
                            y_sb[:, sb4, ts(df, 512)], yps[df][:],
                            y_sb[:, sb4, ts(df, 512)],
                        )

            def q_p(m, i):
                if i == 0:
                    pstate[("q", m)] = aux([P, 512])
                qps = pstate[("q", m)]
                for k in (2 * i, 2 * i + 1):
                    nc.tensor.matmul(
                        qps[:], wq_sb[:, k, ts(m, P)], xc0[:, k, :],
                        start=(k == 0), stop=(k == KC - 1),
                    )

            def q_fin(m):
                qps = pstate.pop(("q", m))
                nc.vector.tensor_copy(qz[0:DK, m, 0, :], qps[0:DK, :])
                nc.vector.tensor_copy(qz[DK:P, m, 1, :], qps[DK:P, :])

            # hook schedule: {pass: {tb: [closures]}} -- one <=1us piece per
            # tb so the PE never bursts past the exp cadence.  Deadlines:
            # K2T block c by sc(4c) emission; Vp block c by av(4c) (tb 4c+4,
            # attn@V lag 4); qz m by the next pass's sc(0).
            hooks = {
                0: {1: [lambda: k2_a(1)], 2: [lambda: k2_b(1)],
                    3: [lambda: v_a(1)], 4: [lambda: v_b(1)],
                    5: [lambda: v_tr(1)],
                    6: [lambda: k2_a(2)], 7: [lambda: k2_b(2)],
                    8: [lambda: v_a(2)], 9: [lambda: v_b(2)],
                    10: [lambda: v_tr(2), lambda: k2_a(3)],
                    11: [lambda: k2_b(3)],
                    12: [lambda: v_a(3)], 13: [lambda: v_b(3)],
                    14: [lambda: v_tr(3)]},
            }
            for p in range(1, 7):
                for i in range(4):
                    hooks.setdefault(p, {}).setdefault(10 + i, []).append(
                        lambda m=p + 1, i=i: q_p(m, i)
                    )
                hooks.setdefault(p, {}).setdefault(14, []).append(
                    lambda m=p + 1: q_fin(m)
                )

            prev = None  # (hp, acc, tail exs 13..15)
            for hp in range(H2):
                acc = ps.tile([96, 2, SPB], fp32, name="acc", tag="acc",
                              bufs=1)
                exs = {}
                oT_prev = None
                for tb in range(NT):
                    sc = ps.tile([P, 2, SPB], fp32, name=f"sc{tb % 2}",
                                 tag=f"sc{tb % 2}", bufs=1)
                    for j in range(2):
                        nc.tensor.matmul(
                            sc[:, j, :], K2T[:, ts(tb, P)], qz[:, hp, j, :],
                            start=True, stop=True,
                        )
                    # attn@V runs 4 tb behind exp (ex bufs=4); emitted before
                    # the activation so the freed ex slot is ready in time.
                    # The previous pass's last four attn@V are spread over
                    # its own tb15 (av 12) and this pass's tb0/tb1 so no
                    # single tb block overloads the PE.
                    if tb == 0 and prev is not None:
                        emit_av(prev[1], 13, prev[2].pop(13))
                    if tb == 1 and prev is not None:
                        emit_av(prev[1], 14, prev[2].pop(14))
                    if tb >= 4:
                        emit_av(acc, tb - 4, exs.pop(tb - 4))
                    ex = sb.tile([P, 2, SPB], mdt, name="ex", tag="ex", bufs=4)
                    nc.scalar.activation(ex[:], sc[:], Act.Exp, scale=scale)
                    exs[tb] = ex
                    if tb == 1 and prev is not None:
                        emit_av(prev[1], 15, prev[2].pop(15))
                        # evacuate the finished accumulator so this pass's
                        # attn@V can claim the psum banks at tb4; the
                        # normalize then works from the SBUF copy at leisure
                        nc.vector.tensor_copy(acc_sb[:], prev[1][:])
                    if tb == 15:
                        emit_av(acc, 12, exs.pop(12))
                    if prev is not None:
                        if tb == 2:
                            norm_a(prev[0])
                        elif tb == 3:
                            oT_prev = norm_b(prev[0])
                        elif 7 <= tb <= 10:
                            emit_y(prev[0], oT_prev, tb - 7)
                    for fn in hooks.get(hp, {}).get(tb, []):
                        fn()
                prev = (hp, acc, exs)

            # tail: last pass's deferred attn@V + normalize + y + writeback.
            # y partials rotate through the dead sc banks as well as aux so
            # the four output blocks pipeline instead of serializing; the
            # final adds cast to a bf16 output buffer, halving writeback DMA.
            y_out = sb.tile([P, NSB, D], bf16, name="y_out")
            for tb in range(13, NT):
                emit_av(prev[1], tb, prev[2].pop(tb))
            # tail normalize: only rows 0:32 (which hold the denominator)
            # gate the reciprocal; the V rows follow while it runs
            nc.vector.tensor_copy(acc_sb[0:32, :, :], prev[1][0:32, :, :])
            norm_a(prev[0])
            nc.vector.tensor_copy(
                acc_sb[32:64, :, :], prev[1][32:64, :, :]
            )
            nc.vector.tensor_copy(
                acc_sb[DK:96, :, :], prev[1][DK:96, :, :]
            )
            oT_last = norm_b(prev[0], tail=True)
            for sb4 in range(NSB):
                yps = [
                    ps.tile([P, 512], fp32, name="typs", tag=f"sc{sb4 % 2}",
                            bufs=1),
                    aux([P, 512]),
                ]
                for df in range(2):
                    nc.tensor.matmul(
                        yps[df][:], oT_last[:, ts(sb4, P)],
                        wo2_sb[:, H2 - 1, ts(df, 512)],
                        start=True, stop=True,
                    )
                for df in range(2):
                    nc.vector.tensor_add(
                        y_out[:, sb4, ts(df, 512)], yps[df][:],
                        y_sb[:, sb4, ts(df, 512)],
                    )
                # issue from the Scalar engine: idle after the last exp, and
                # far cheaper per issue than gpsimd
                nc.scalar.dma_start(y[ts(sb4, P), :], y_out[:, sb4, :])

    nc.compile()
    return nc


def make_in_maps(x, w_q, w_k, w_v, w_out):
    import ml_dtypes

    bf16 = ml_dtypes.bfloat16
    cst = np.zeros((P, 256), dtype=np.float32)
    cst[0:DK, 0:DK] = np.eye(DK, dtype=np.float32)
    cst[:, DK:192] = 1.0
    x = np.ascontiguousarray(np.asarray(x, dtype=np.float32))
    w_q = np.asarray(w_q, dtype=np.float32)
    w_k = np.asarray(w_k, dtype=np.float32)
    w_v = np.asarray(w_v, dtype=np.float32)
    w_out = np.asarray(w_out, dtype=np.float32)

    wqT = np.ascontiguousarray(w_q.T.astype(bf16))
    wkkT = np.ascontiguousarray(
        np.concatenate([w_k.T, w_k.T], axis=1).astype(bf16)
    )
    wvT = np.ascontiguousarray(
        np.concatenate([w_v.T, np.zeros((D, 1), np.float32)], axis=1)
        .astype(bf16)
    )
    # head-pair-stacked w_out.T: wo2[phi*64+dv, hp, d] = w_out.T[(2hp+phi)*64+dv, d]
    wo2 = np.ascontiguousarray(
        w_out.T.reshape(H2, 2, DK, D).transpose(1, 2, 0, 3).reshape(P, H2, D)
    )

    in_maps = []
    for c in range(NCORES):
        b, r = divmod(c, GPB)
        # roll this core's query rows to the front; t-order is irrelevant
        # (attention sums over t), so K/V are unaffected
        xb = np.roll(x[b], -r * SPB, axis=0)
        xTc = np.ascontiguousarray(xb.T.astype(bf16))
        in_maps.append(
            {"xT": xTc, "wqT": wqT, "wkkT": wkkT, "wvT": wvT, "wo2": wo2,
             "cst": cst}
        )
    return in_maps


_BUILD_CACHE = {}


def _cached_nc(scale: float):
    key = round(float(scale), 12)
    if key not in _BUILD_CACHE:
        _BUILD_CACHE[key] = build_bass(float(scale))
    return _BUILD_CACHE[key]


def run_on_hw(in_maps, scale, trace=False):
    from concourse.bass_utils import run_bass_kernel_spmd

    nc = _cached_nc(scale)
    return run_bass_kernel_spmd(nc, in_maps, list(range(NCORES)), trace=trace)


def assemble(results):
    out = np.empty((B, S, D), dtype=np.float32)
    for c in range(NCORES):
        b, r = divmod(c, GPB)
        out[b, r * SPB:(r + 1) * SPB] = results[c]["y"].astype(np.float32)
    return out


def kernel(x, w_q, w_k, w_v, w_out, softmax_scale):
    scale = float(np.asarray(softmax_scale).reshape(-1)[0])
    in_maps = make_in_maps(x, w_q, w_k, w_v, w_out)
    res = run_on_hw(in_maps, scale, trace=False)
    return assemble(res.results)
